# revision 21
# baseline (speedup 1.0000x reference)
"""Trainium2 Bass kernel for nn_DLRMPrefetcher (8 NeuronCores, SPMD).

Strategy:
- Data-parallel the two small transformers over batch (16 rows/core).
- Activations kept TRANSPOSED on device: xT [D=128 partitions, tokens free].
- Embedding lookup via indirect DMA gather (int32 row indices) + PE transpose.
- Attention per (seq, head) with transposed scores [k, q]; softmax denominator
  folded into the AV matmul via a ones-column appended to V.
- One AllGather (gpsimd collective) of the per-core hidden state h [512 x 16].
- Output projections vocab-sharded: each core computes [1024, 12500] of the idx
  logits and [1024, 125] of the tab logits; host concatenates shards.
- bf16 compute on PE, fp32 PSUM accumulation, fp32 output assembly.
"""
import numpy as np
import ml_dtypes

import concourse.bass as bass
import concourse.mybir as mybir
import concourse.bacc as bacc
import concourse.tile as tile
from concourse.bass_utils import run_bass_kernel_spmd
from concourse.masks import make_identity

dt = mybir.dt
AF = mybir.ActivationFunctionType
OP = mybir.AluOpType

# ---- model constants (hardcoded per problem spec) ----
P = 128
B, S = 128, 200
D, H, HD = 128, 8, 16
LAYERS = 2
DFF = 2048
HID = 4 * D           # 512
TOUT = 8
TBLV = 1000
IDXV = 100000
IDX_ROWS = 100352     # NSEG * BLK
EPS = 1e-5
NCORE = 8
BL = B // NCORE       # 16 rows per core
NSEQ = BL             # sequences per encoder per core
NPAIR = NSEQ // 2     # 8 pairs (2 seqs of 200 -> 400 cols per pair)
NCHUNK = NSEQ * S // P  # 25 gather chunks of 128 tokens
VSH_I = IDXV // NCORE   # 12500
VSH_T = TBLV // NCORE   # 125
VT = 500                # idx proj vocab tile (N of matmul)
NVT = VSH_I // VT       # 25
KCH = [(0, 128), (128, 72)]   # k-position chunks within one sequence
DFC = DFF // P          # 16 dff chunks

_CACHE = {}


def _build_nc():
    nc = bacc.Bacc("TRN2", target_bir_lowering=False, debug=False,
                   enable_asserts=False, num_devices=NCORE)

    def din(name, shape, d):
        return nc.dram_tensor(name, shape, d, kind="ExternalInput")

    temb = din("temb", [TBLV, D], dt.float32)
    iemb = din("iemb", [IDX_ROWS, D], dt.float32)
    tidx = din("tidx", [P, NCHUNK], dt.int32)
    iidx = din("iidx", [P, NCHUNK], dt.int32)
    # per (e*2+l): lhsT layouts
    wqT = din("wqT", [4, P, 4, P], dt.bfloat16)  # head-padded
    wkT = din("wkT", [4, P, 4, P], dt.bfloat16)
    wvT = din("wvT", [4, P, D], dt.bfloat16)
    woT = din("woT", [4, P, D], dt.bfloat16)
    w1T = din("w1T", [4, P, DFC, P], dt.bfloat16)   # [:, :, fc, :] = W1.T cols chunk
    w2T = din("w2T", [4, P, DFC, P], dt.bfloat16)   # [:, :, fc, :] = W2.T rows chunk
    bq = din("bq", [4, P, 4], dt.float32)
    bk = din("bk", [4, P, 4], dt.float32)
    boe = din("boe", [4, P, 1], dt.float32)          # bo + Wo @ bv
    b1 = din("b1", [4, P, DFC], dt.float32)
    b2 = din("b2", [4, P, 1], dt.float32)
    lnp = din("lnp", [4, P, 4], dt.float32)          # cols: g1, b1, g2, b2
    wlinT = din("wlinT", [P, 2, HID], dt.bfloat16)   # [:, c, :] = Wlin.T rows chunk c
    blin = din("blin", [P, 4], dt.float32)
    wtabT = din("wtabT", [P, 4, VSH_T], dt.bfloat16)
    widxT = din("widxT", [HID, VSH_I], dt.bfloat16)
    tvec = din("tvec", [2, VSH_T], dt.float32)       # row0 bias, row1 wlast
    ivec = din("ivec", [2, VSH_I], dt.float32)
    onesb = din("onesb", [P, 1], dt.bfloat16)

    otab = nc.dram_tensor("otab", [P, TOUT, VSH_T], dt.float32, kind="ExternalOutput")
    oidx = nc.dram_tensor("oidx", [P, TOUT, VSH_I], dt.float32, kind="ExternalOutput")

    # collective buffers
    cc_in = nc.dram_tensor("cc_in", [P, 4, BL], dt.bfloat16)
    cc_out = nc.dram_tensor("cc_out", [NCORE, P, 4, BL], dt.bfloat16, addr_space="Shared")

    with tile.TileContext(nc) as tc:
        with tc.tile_pool(name="w", bufs=1) as sw, \
             tc.tile_pool(name="s", bufs=3) as ss, \
             tc.tile_pool(name="pp", bufs=2) as sp, \
             tc.tile_pool(name="psA", bufs=4, space="PSUM") as psA, \
             tc.tile_pool(name="psB", bufs=2, space="PSUM") as psB:

            # ---- load persistent weights ----
            def ld(src_ap, shape, d, tag):
                t = sw.tile(shape, d, tag=tag, name=tag)
                nc.sync.dma_start(out=t[:], in_=src_ap)
                return t

            ident = sw.tile([P, P], dt.float32, tag="ident")
            make_identity(nc, ident[:])
            eps_t = sw.tile([1, 1], dt.float32, tag="eps")
            nc.vector.memset(eps_t[:], EPS)
            ones1 = sw.tile([1, P], dt.float32, tag="ones1")
            nc.vector.memset(ones1[:], 1.0)
            identb = sw.tile([P, P], dt.bfloat16, tag="identb")
            make_identity(nc, identb[:])
            ones_sb = ld(onesb[:], [P, 1], dt.bfloat16, "ones")

            wq_t, wk_t, wv_t, wo_t, w1_t, w2_t = [], [], [], [], [], []
            bq_t, bk_t, bo_t, b1_t, b2_t, ln_t = [], [], [], [], [], []
            for p4 in range(4):
                wq_t.append(ld(wqT[p4], [P, 4, P], dt.bfloat16, f"wq{p4}"))
                wk_t.append(ld(wkT[p4], [P, 4, P], dt.bfloat16, f"wk{p4}"))
                wv_t.append(ld(wvT[p4], [P, D], dt.bfloat16, f"wv{p4}"))
                wo_t.append(ld(woT[p4], [P, D], dt.bfloat16, f"wo{p4}"))
                w1_t.append(ld(w1T[p4], [P, DFC, P], dt.bfloat16, f"w1{p4}"))
                w2_t.append(ld(w2T[p4], [P, DFC, P], dt.bfloat16, f"w2{p4}"))
                bq_t.append(ld(bq[p4], [P, 4], dt.float32, f"bq{p4}"))
                bk_t.append(ld(bk[p4], [P, 4], dt.float32, f"bk{p4}"))
                bo_t.append(ld(boe[p4], [P, 1], dt.float32, f"bo{p4}"))
                b1_t.append(ld(b1[p4], [P, DFC], dt.float32, f"b1{p4}"))
                b2_t.append(ld(b2[p4], [P, 1], dt.float32, f"b2{p4}"))
                ln_t.append(ld(lnp[p4], [P, 4], dt.float32, f"ln{p4}"))
            wlin_t = ld(wlinT[:], [P, 2, HID], dt.bfloat16, "wlin")
            blin_t = ld(blin[:], [P, 4], dt.float32, "blin")
            wtab_t = ld(wtabT[:], [P, 4, VSH_T], dt.bfloat16, "wtab")

            # ---- embeddings: gather + transpose into xT[enc] [128, 3200] bf16 ----
            xT = [sw.tile([P, NSEQ * S], dt.bfloat16, tag=f"xT{e}", name=f"xT{e}") for e in range(2)]
            for enc, (emb, idxs) in enumerate(((temb, tidx), (iemb, iidx))):
                idx_sb = sw.tile([P, NCHUNK], dt.int32, tag=f"idx{enc}")
                nc.sync.dma_start(out=idx_sb[:], in_=idxs[:])
                for c in range(NCHUNK):
                    gx = ss.tile([P, D], dt.float32, tag="gx")
                    nc.gpsimd.indirect_dma_start(
                        out=gx[:], out_offset=None, in_=emb[:],
                        in_offset=bass.IndirectOffsetOnAxis(ap=idx_sb[:, c:c + 1], axis=0))
                    xp = psA.tile([P, P], dt.float32, tag="ps")
                    nc.tensor.transpose(out=xp[:], in_=gx[:], identity=ident[:])
                    nc.scalar.copy(out=xT[enc][:, c * P:(c + 1) * P], in_=xp[:])

            # means accumulators [128, BL] f32
            mean_t = [sw.tile([P, BL], dt.float32, tag=f"mean{e}", name=f"mean{e}") for e in range(2)]

            # ---- transformer ----
            for enc in range(2):
                x = xT[enc]
                for l in range(LAYERS):
                    p4 = enc * 2 + l
                    last = (l == LAYERS - 1)
                    for pair in range(NPAIR):
                        c0 = pair * 2 * S
                        W = 2 * S  # 400
                        xs = x[:, c0:c0 + W]
                        # qT, kT in head-padded tiles: tile ti holds heads 2ti
                        # (partitions 0:16) and 2ti+1 (partitions 64:80)
                        qTl, kTl = [], []
                        for ti in range(4):
                            q_ps = psA.tile([P, W], dt.float32, tag="ps")
                            nc.tensor.matmul(out=q_ps[:], lhsT=wq_t[p4][:, ti, :],
                                             rhs=xs, start=True, stop=True)
                            qT = ss.tile([P, W], dt.bfloat16, tag=f"qT{ti}",
                                         name=f"qT{ti}")
                            nc.scalar.activation(out=qT[:], in_=q_ps[:],
                                                 func=AF.Identity,
                                                 bias=bq_t[p4][:, ti:ti + 1], scale=1.0)
                            qTl.append(qT)
                            k_ps = psA.tile([P, W], dt.float32, tag="ps")
                            nc.tensor.matmul(out=k_ps[:], lhsT=wk_t[p4][:, ti, :],
                                             rhs=xs, start=True, stop=True)
                            kT = ss.tile([P, W], dt.bfloat16, tag=f"kT{ti}",
                                         name=f"kT{ti}")
                            nc.scalar.activation(out=kT[:], in_=k_ps[:],
                                                 func=AF.Identity,
                                                 bias=bk_t[p4][:, ti:ti + 1], scale=1.0)
                            kTl.append(kT)
                        # v natural layout + ones col: va[j][kc] [<=128, 8, 18]
                        va = {}
                        for j in range(2):
                            for kc, (ko, kn) in enumerate(KCH):
                                v_ps = psA.tile([P, D], dt.float32, tag="ps")
                                nc.tensor.matmul(
                                    out=v_ps[:kn, :],
                                    lhsT=x[:, c0 + j * S + ko: c0 + j * S + ko + kn],
                                    rhs=wv_t[p4][:], start=True, stop=True)
                                vt = ss.tile([P, H, HD + 2], dt.bfloat16,
                                             tag=f"va{j}{kc}", name=f"va{j}{kc}")
                                nc.vector.tensor_copy(
                                    out=vt[:kn, :, 0:HD],
                                    in_=v_ps[:kn, :].rearrange("p (h d) -> p h d", h=H))
                                nc.vector.memset(vt[:kn, :, HD:HD + 1], 1.0)
                                va[(j, kc)] = vt
                        # attention per (seq, head); o assembled in NATURAL layout
                        # (heads along free dim), normalized, then transposed.
                        oT = ss.tile([P, W], dt.bfloat16, tag="oT")
                        for j in range(2):
                            el = {}
                            for h in range(H):
                                ti, hb = h // 2, 64 * (h % 2)
                                hps = slice(hb, hb + HD)
                                for kc, (ko, kn) in enumerate(KCH):
                                    sc = psA.tile([P, S], dt.float32, tag="ps")
                                    nc.tensor.matmul(
                                        out=sc[:kn, :],
                                        lhsT=kTl[ti][hps, j * S + ko: j * S + ko + kn],
                                        rhs=qTl[ti][hps, j * S: (j + 1) * S],
                                        start=True, stop=True)
                                    e = ss.tile([P, S], dt.bfloat16, tag=f"e{h}{kc}",
                                                name=f"e{h}{kc}")
                                    nc.scalar.activation(out=e[:kn, :], in_=sc[:kn, :],
                                                         func=AF.Exp, scale=0.25)
                                    el[(h, kc)] = e
                            # o_buf[q, 17h:17h+17] = sum_k e_h[k,q] * [v_h | 1][k,:]
                            for qc, (qo, qn) in enumerate(KCH):
                                ob = psA.tile([P, H * (HD + 1)], dt.float32, tag="ps")
                                for h in range(H):
                                    for kc, (ko, kn) in enumerate(KCH):
                                        nc.tensor.matmul(
                                            out=ob[:qn, h * (HD + 1):(h + 1) * (HD + 1)],
                                            lhsT=el[(h, kc)][:kn, qo:qo + qn],
                                            rhs=va[(j, kc)][:kn, h, 0:HD + 1],
                                            start=(kc == 0), stop=(kc == 1))
                                # extract denominators (free-dim strided), normalize
                                rcq = ss.tile([P, H], dt.float32, tag="rcq")
                                dnq = ss.tile([P, H], dt.float32, tag="dnq")
                                nc.vector.tensor_copy(
                                    out=dnq[:qn, :],
                                    in_=ob[:qn, :].rearrange(
                                        "p (h d) -> p h d", h=H)[:, :, HD:HD + 1].squeeze())
                                nc.vector.reciprocal(out=rcq[:qn, :], in_=dnq[:qn, :])
                                onat = ss.tile([P, D], dt.bfloat16, tag="onat")
                                nc.vector.tensor_tensor(
                                    out=onat[:qn, :].rearrange("p (h d) -> p h d", h=H),
                                    in0=ob[:qn, :].rearrange(
                                        "p (h d) -> p h d", h=H)[:, :, 0:HD],
                                    in1=rcq[:qn, :].unsqueeze(2).broadcast_to(
                                        [qn, H, HD]),
                                    op=OP.mult)
                                # transpose back to oT columns
                                otp = psB.tile([P, P], dt.bfloat16, tag="pst")
                                nc.tensor.transpose(out=otp[:, :qn], in_=onat[:qn, :],
                                                    identity=identb[:qn, :qn])
                                nc.scalar.copy(out=oT[:, j * S + qo: j * S + qo + qn],
                                               in_=otp[:, :qn])
                        # Wo + residual
                        y_ps = psA.tile([P, W], dt.float32, tag="ps")
                        nc.tensor.matmul(out=y_ps[:], lhsT=wo_t[p4][:], rhs=oT[:],
                                         start=True, stop=True)
                        yb = ss.tile([P, W], dt.bfloat16, tag="yb")
                        nc.scalar.activation(out=yb[:], in_=y_ps[:], func=AF.Identity,
                                             bias=bo_t[p4][:, 0:1], scale=1.0)
                        x1 = ss.tile([P, W], dt.bfloat16, tag="x1")
                        nc.vector.tensor_tensor(out=x1[:], in0=xs, in1=yb[:], op=OP.add)

                        def layer_norm(xin, g_ap, b_ap, out_ap, accum_cols=None):
                            # stats over partition dim via ones-matmul
                            sq = ss.tile([P, W], dt.bfloat16, tag="sq")
                            nc.vector.tensor_tensor(out=sq[:], in0=xin[:], in1=xin[:],
                                                    op=OP.mult)
                            s1 = psA.tile([1, W], dt.float32, tag="ps")
                            nc.tensor.matmul(out=s1[:], lhsT=ones_sb[:], rhs=xin[:],
                                             start=True, stop=True)
                            s2 = psA.tile([1, W], dt.float32, tag="ps")
                            nc.tensor.matmul(out=s2[:], lhsT=ones_sb[:], rhs=sq[:],
                                             start=True, stop=True)
                            m = ss.tile([1, W], dt.float32, tag="m")
                            nc.scalar.activation(out=m[:], in_=s1[:], func=AF.Copy,
                                                 scale=1.0 / D)
                            msq = ss.tile([1, W], dt.float32, tag="msq")
                            nc.scalar.activation(out=msq[:], in_=s2[:], func=AF.Copy,
                                                 scale=1.0 / D)
                            m2 = ss.tile([1, W], dt.float32, tag="m2")
                            nc.vector.tensor_tensor(out=m2[:], in0=m[:], in1=m[:],
                                                    op=OP.mult)
                            var = ss.tile([1, W], dt.float32, tag="var")
                            nc.vector.tensor_tensor(out=var[:], in0=msq[:], in1=m2[:],
                                                    op=OP.subtract)
                            std = ss.tile([1, W], dt.float32, tag="std")
                            nc.scalar.activation(out=std[:], in_=var[:], func=AF.Sqrt,
                                                 bias=eps_t[0:1, 0:1], scale=1.0)
                            rstd = ss.tile([1, W], dt.float32, tag="rstd")
                            nc.vector.reciprocal(out=rstd[:], in_=std[:])
                            mrep = psA.tile([P, W], dt.float32, tag="ps")
                            nc.tensor.matmul(out=mrep[:], lhsT=ones1[:], rhs=m[:],
                                             start=True, stop=True)
                            rrep = psA.tile([P, W], dt.float32, tag="ps")
                            nc.tensor.matmul(out=rrep[:], lhsT=ones1[:], rhs=rstd[:],
                                             start=True, stop=True)
                            t0 = ss.tile([P, W], dt.bfloat16, tag="t0")
                            nc.vector.tensor_tensor(out=t0[:], in0=xin[:],
                                                    in1=mrep[:], op=OP.subtract)
                            t1 = ss.tile([P, W], dt.bfloat16, tag="t1")
                            nc.vector.tensor_tensor(out=t1[:], in0=t0[:],
                                                    in1=rrep[:], op=OP.mult)
                            if accum_cols is None:
                                nc.scalar.activation(out=out_ap, in_=t1[:],
                                                     func=AF.Identity, bias=b_ap,
                                                     scale=g_ap)
                            else:
                                for j2 in range(2):
                                    nc.scalar.activation(
                                        out=out_ap[:, j2 * S:(j2 + 1) * S],
                                        in_=t1[:, j2 * S:(j2 + 1) * S],
                                        func=AF.Identity, bias=b_ap, scale=g_ap,
                                        accum_out=accum_cols[j2])

                        xn = ss.tile([P, W], dt.bfloat16, tag="xn")
                        layer_norm(x1, ln_t[p4][:, 0:1], ln_t[p4][:, 1:2], xn[:])
                        # FFN
                        f2_ps = psB.tile([P, W], dt.float32, tag="acc")
                        for fc in range(DFC):
                            f1_ps = psA.tile([P, W], dt.float32, tag="ps")
                            nc.tensor.matmul(out=f1_ps[:], lhsT=w1_t[p4][:, fc, :],
                                             rhs=xn[:], start=True, stop=True)
                            f1b = ss.tile([P, W], dt.bfloat16, tag="f1b")
                            nc.scalar.activation(out=f1b[:], in_=f1_ps[:], func=AF.Relu,
                                                 bias=b1_t[p4][:, fc:fc + 1], scale=1.0)
                            nc.tensor.matmul(out=f2_ps[:], lhsT=w2_t[p4][:, fc, :],
                                             rhs=f1b[:], start=(fc == 0),
                                             stop=(fc == DFC - 1))
                        fb = ss.tile([P, W], dt.bfloat16, tag="fb")
                        nc.scalar.activation(out=fb[:], in_=f2_ps[:], func=AF.Identity,
                                             bias=b2_t[p4][:, 0:1], scale=1.0)
                        x2 = ss.tile([P, W], dt.bfloat16, tag="x2")
                        nc.vector.tensor_tensor(out=x2[:], in0=xn[:], in1=fb[:], op=OP.add)
                        if not last:
                            layer_norm(x2, ln_t[p4][:, 2:3], ln_t[p4][:, 3:4], xs)
                        else:
                            rows = (2 * pair, 2 * pair + 1)
                            layer_norm(
                                x2, ln_t[p4][:, 2:3], ln_t[p4][:, 3:4], xs,
                                accum_cols=[mean_t[enc][:, r:r + 1] for r in rows])

            # ---- MLP head: hT [512, BL] -> cc_in ----
            mb = []
            for e in range(2):
                t = ss.tile([P, BL], dt.bfloat16, tag=f"mb{e}", name=f"mb{e}")
                nc.vector.tensor_copy(out=t[:], in_=mean_t[e][:])
                mb.append(t)
            hloc = ss.tile([P, 4, BL], dt.bfloat16, tag="hloc")
            for m in range(4):
                h_ps = psA.tile([P, BL], dt.float32, tag="ps")
                for c in range(2):
                    nc.tensor.matmul(out=h_ps[:],
                                     lhsT=wlin_t[:, c, m * P:(m + 1) * P],
                                     rhs=mb[c][:], start=(c == 0), stop=(c == 1))
                nc.scalar.activation(out=hloc[:, m, :], in_=h_ps[:], func=AF.Relu,
                                     bias=blin_t[:, m:m + 1], scale=1.0 / S)
            nc.sync.dma_start(out=cc_in[:], in_=hloc[:])
            nc.gpsimd.collective_compute(
                "AllGather", OP.bypass,
                replica_groups=[list(range(NCORE))],
                ins=[cc_in[:]], outs=[cc_out[:]])
            hT = sw.tile([P, 4, B], dt.bfloat16, tag="hT")
            for r in range(NCORE):
                for kk in range(4):
                    nc.sync.dma_start(out=hT[:, kk, r * BL:(r + 1) * BL],
                                      in_=cc_out[r, :, kk, :])

            # ---- projections ----
            def proj(nvt, vtw, w_tiles_fn, vec_dram, out_dram):
                for vt_i in range(nvt):
                    v0 = vt_i * vtw
                    vb = sp.tile([1, vtw], dt.float32, tag="vb", name="vb")
                    nc.sync.dma_start(out=vb[:], in_=vec_dram[0:1, v0:v0 + vtw])
                    vl = sp.tile([1, vtw], dt.float32, tag="vl", name="vl")
                    nc.sync.dma_start(out=vl[:], in_=vec_dram[1:2, v0:v0 + vtw])
                    vbr_ps = psA.tile([P, vtw], dt.float32, tag="ps")
                    nc.tensor.matmul(out=vbr_ps[:], lhsT=ones1[:], rhs=vb[:],
                                     start=True, stop=True)
                    vbr = sp.tile([P, vtw], dt.float32, tag="vbr", name="vbr")
                    nc.scalar.copy(out=vbr[:], in_=vbr_ps[:])
                    vlr_ps = psA.tile([P, vtw], dt.float32, tag="ps")
                    nc.tensor.matmul(out=vlr_ps[:], lhsT=ones1[:], rhs=vl[:],
                                     start=True, stop=True)
                    vlr = sp.tile([P, vtw], dt.float32, tag="vlr", name="vlr")
                    nc.scalar.copy(out=vlr[:], in_=vlr_ps[:])
                    bp = psA.tile([P, vtw], dt.float32, tag="ps")
                    for kk in range(4):
                        nc.tensor.matmul(out=bp[:], lhsT=hT[:, kk, :],
                                         rhs=w_tiles_fn(kk, v0, vtw),
                                         start=(kk == 0), stop=(kk == 3))
                    prev = None
                    for t in range(TOUT):
                        ot = sp.tile([P, vtw], dt.float32, tag=f"o{t}", name=f"o{t}")
                        nc.vector.tensor_tensor(
                            out=ot[:], in0=(bp[:] if t == 0 else prev[:]),
                            in1=(vbr if t == 0 else vlr)[:, :], op=OP.add)
                        nc.sync.dma_start(out=out_dram[:, t, v0:v0 + vtw], in_=ot[:])
                        prev = ot

            def wtab_fn(kk, v0, vtw):
                return wtab_t[:, kk, v0:v0 + vtw].squeeze()

            def widx_fn(kk, v0, vtw):
                wt = sp.tile([P, vtw], dt.bfloat16, tag="wvt")
                nc.sync.dma_start(out=wt[:], in_=widxT[kk * P:(kk + 1) * P, v0:v0 + vtw])
                return wt[:]

            proj(1, VSH_T, wtab_fn, tvec, otab)
            proj(NVT, VT, widx_fn, ivec, oidx)

    nc.compile()
    return nc


def _prep(inputs):
    f32 = np.float32
    bf = ml_dtypes.bfloat16
    g = {k: np.asarray(v) for k, v in inputs.items()}

    Wqkv, bqkv = g["Wqkv"], g["bqkv"]
    Wo, bo = g["Wo"], g["bo"]
    W1, b1_, W2, b2_ = g["W1"], g["b1"], g["W2"], g["b2"]

    wqT = np.zeros((4, P, 4, P), f32); wkT = np.zeros((4, P, 4, P), f32)
    wvT = np.zeros((4, P, D), f32); woT = np.zeros((4, P, D), f32)
    w1T = np.zeros((4, P, DFC, P), f32); w2T = np.zeros((4, P, DFC, P), f32)
    bq = np.zeros((4, P, 4), f32); bk = np.zeros((4, P, 4), f32)
    boe = np.zeros((4, P, 1), f32); b1h = np.zeros((4, P, DFC), f32)
    b2h = np.zeros((4, P, 1), f32); lnp = np.zeros((4, P, 4), f32)
    for e in range(2):
        for l in range(LAYERS):
            p4 = e * 2 + l
            WqTf = Wqkv[e, l, 0:D, :].T       # [in-D, out-D]
            WkTf = Wqkv[e, l, D:2 * D, :].T
            for ti in range(4):
                for s2, h in ((0, 2 * ti), (64, 2 * ti + 1)):
                    wqT[p4, :, ti, s2:s2 + HD] = WqTf[:, h * HD:(h + 1) * HD]
                    wkT[p4, :, ti, s2:s2 + HD] = WkTf[:, h * HD:(h + 1) * HD]
                    bq[p4, s2:s2 + HD, ti] = bqkv[e, l, h * HD:(h + 1) * HD]
                    bk[p4, s2:s2 + HD, ti] = bqkv[e, l, D + h * HD:D + (h + 1) * HD]
            wvT[p4] = Wqkv[e, l, 2 * D:3 * D, :].T
            woT[p4] = Wo[e, l].T
            w1T[p4] = W1[e, l].T.reshape(P, DFC, P)
            w2T[p4] = W2[e, l].T.reshape(DFC, P, P).transpose(1, 0, 2)
            bv = bqkv[e, l, 2 * D:3 * D]
            boe[p4, :, 0] = bo[e, l] + Wo[e, l] @ bv
            b1h[p4] = b1_[e, l].reshape(DFC, P).T
            b2h[p4, :, 0] = b2_[e, l]
            lnp[p4, :, 0] = g["ln1_g"][e, l]; lnp[p4, :, 1] = g["ln1_b"][e, l]
            lnp[p4, :, 2] = g["ln2_g"][e, l]; lnp[p4, :, 3] = g["ln2_b"][e, l]

    wlinT = g["Wlin"].T.reshape(2, P, HID).transpose(1, 0, 2).copy()  # [128,2,512]
    blin = g["blin"].reshape(4, P).T                  # [128, 4]
    Wtab, btab = g["Wtab"], g["btab"]
    Widx, bidx = g["Widx"], g["bidx"]
    onesb = np.ones((P, 1), f32)

    in_maps = []
    for c in range(NCORE):
        rows = slice(c * BL, (c + 1) * BL)
        tids = g["table_id_seq"][rows].reshape(-1)     # [3200]
        iids = g["idx_id_seq"][rows].reshape(-1)
        tsh = slice(c * VSH_T, (c + 1) * VSH_T)
        ish = slice(c * VSH_I, (c + 1) * VSH_I)
        WtabS, WidxS = Wtab[tsh], Widx[ish]
        tvec = np.stack([btab[tsh], WtabS[:, HID]]).astype(f32)
        ivec = np.stack([bidx[ish], WidxS[:, HID]]).astype(f32)
        in_maps.append({
            "temb": g["table_embed"].astype(f32),
            "iemb": g["idx_embed"].astype(f32),
            "tidx": tids.reshape(NCHUNK, P).T.copy().astype(np.int32),
            "iidx": iids.reshape(NCHUNK, P).T.copy().astype(np.int32),
            "wqT": wqT.astype(bf), "wkT": wkT.astype(bf),
            "wvT": wvT.astype(bf), "woT": woT.astype(bf),
            "w1T": w1T.astype(bf), "w2T": w2T.astype(bf),
            "bq": bq, "bk": bk, "boe": boe, "b1": b1h, "b2": b2h, "lnp": lnp,
            "wlinT": wlinT.astype(bf), "blin": blin.astype(f32),
            "wtabT": WtabS[:, :HID].T.reshape(4, P, VSH_T).transpose(1, 0, 2).copy().astype(bf),
            "widxT": WidxS[:, :HID].T.copy().astype(bf),
            "tvec": tvec, "ivec": ivec,
            "onesb": onesb.astype(bf),
        })
    return in_maps


def kernel(**inputs):
    if "nc" not in _CACHE:
        _CACHE["nc"] = _build_nc()
    nc = _CACHE["nc"]
    in_maps = _prep(inputs)
    res = run_bass_kernel_spmd(nc, in_maps, core_ids=list(range(NCORE)))
    _CACHE["last_res"] = res
    tab = np.concatenate(
        [res.results[c]["otab"].reshape(B * TOUT, VSH_T) for c in range(NCORE)], axis=1)
    idx = np.concatenate(
        [res.results[c]["oidx"].reshape(B * TOUT, VSH_I) for c in range(NCORE)], axis=1)
    return tab, idx


# revision 23
# speedup vs baseline: 11873.1869x; 11873.1869x over previous
"""Trainium2 Bass kernel for nn_DLRMPrefetcher (8 NeuronCores, SPMD).

Strategy:
- Data-parallel the two small transformers over batch (16 rows/core).
- Activations kept TRANSPOSED on device: xT [D=128 partitions, tokens free].
- Embedding lookup via indirect DMA gather (int32 row indices) + PE transpose.
- Attention per (seq, head) with transposed scores [k, q]; softmax denominator
  folded into the AV matmul via a ones-column appended to V.
- One AllGather (gpsimd collective) of the per-core hidden state h [512 x 16].
- Output projections vocab-sharded: each core computes [1024, 12500] of the idx
  logits and [1024, 125] of the tab logits; host concatenates shards.
- bf16 compute on PE, fp32 PSUM accumulation, fp32 output assembly.
"""
import sys

for _p in ("/opt/trn_rl_repo", "/opt/pypackages"):
    if _p not in sys.path:
        sys.path.append(_p)

import numpy as np
import ml_dtypes

import concourse.bass as bass
import concourse.mybir as mybir
import concourse.bacc as bacc
import concourse.tile as tile
from concourse.bass_utils import run_bass_kernel_spmd
from concourse.masks import make_identity

dt = mybir.dt
AF = mybir.ActivationFunctionType
OP = mybir.AluOpType

# ---- model constants (hardcoded per problem spec) ----
P = 128
B, S = 128, 200
D, H, HD = 128, 8, 16
LAYERS = 2
DFF = 2048
HID = 4 * D           # 512
TOUT = 8
TBLV = 1000
IDXV = 100000
IDX_ROWS = 100352     # NSEG * BLK
EPS = 1e-5
NCORE = 8
BL = B // NCORE       # 16 rows per core
NSEQ = BL             # sequences per encoder per core
NPAIR = NSEQ // 2     # 8 pairs (2 seqs of 200 -> 400 cols per pair)
NCHUNK = NSEQ * S // P  # 25 gather chunks of 128 tokens
VSH_I = IDXV // NCORE   # 12500
VSH_T = TBLV // NCORE   # 125
VT = 500                # idx proj vocab tile (N of matmul)
NVT = VSH_I // VT       # 25
KCH = [(0, 128), (128, 72)]   # k-position chunks within one sequence
DFC = DFF // P          # 16 dff chunks

_CACHE = {}


def _build_nc(single=False):
    nc = bacc.Bacc("TRN2", target_bir_lowering=False, debug=False,
                   enable_asserts=False, num_devices=(1 if single else NCORE))

    def din(name, shape, d):
        return nc.dram_tensor(name, shape, d, kind="ExternalInput")

    temb = din("temb", [TBLV, D], dt.float32)
    iemb = din("iemb", [IDX_ROWS, D], dt.float32)
    tidx = din("tidx", [P, NCHUNK], dt.int32)
    iidx = din("iidx", [P, NCHUNK], dt.int32)
    # per (e*2+l): lhsT layouts
    wqT = din("wqT", [4, P, 4, P], dt.bfloat16)  # head-padded
    wkT = din("wkT", [4, P, 4, P], dt.bfloat16)
    wvT = din("wvT", [4, P, D], dt.bfloat16)
    woT = din("woT", [4, P, D], dt.bfloat16)
    w1T = din("w1T", [4, P, DFC, P], dt.bfloat16)   # [:, :, fc, :] = W1.T cols chunk
    w2T = din("w2T", [4, P, DFC, P], dt.bfloat16)   # [:, :, fc, :] = W2.T rows chunk
    bq = din("bq", [4, P, 4], dt.float32)
    bk = din("bk", [4, P, 4], dt.float32)
    boe = din("boe", [4, P, 1], dt.float32)          # bo + Wo @ bv
    b1 = din("b1", [4, P, DFC], dt.float32)
    b2 = din("b2", [4, P, 1], dt.float32)
    lnp = din("lnp", [4, P, 4], dt.float32)          # cols: g1, b1, g2, b2
    wlinT = din("wlinT", [P, 2, HID], dt.bfloat16)   # [:, c, :] = Wlin.T rows chunk c
    blin = din("blin", [P, 4], dt.float32)
    wtabT = din("wtabT", [P, 4, VSH_T], dt.bfloat16)
    widxT = din("widxT", [HID, VSH_I], dt.bfloat16)
    tvec = din("tvec", [2, VSH_T], dt.float32)       # row0 bias, row1 wlast
    ivec = din("ivec", [2, VSH_I], dt.float32)
    onesb = din("onesb", [P, 1], dt.bfloat16)

    otab = nc.dram_tensor("otab", [P, TOUT, VSH_T], dt.float32, kind="ExternalOutput")
    oidx = nc.dram_tensor("oidx", [P, TOUT, VSH_I], dt.float32, kind="ExternalOutput")

    # collective buffers
    cc_in = nc.dram_tensor("cc_in", [P, 4, BL], dt.bfloat16)
    cc_out = nc.dram_tensor("cc_out", [NCORE, P, 4, BL], dt.bfloat16, addr_space="Shared")

    with tile.TileContext(nc) as tc:
        with tc.tile_pool(name="w", bufs=1) as sw, \
             tc.tile_pool(name="s", bufs=3) as ss, \
             tc.tile_pool(name="pp", bufs=2) as sp, \
             tc.tile_pool(name="psA", bufs=4, space="PSUM") as psA, \
             tc.tile_pool(name="psB", bufs=2, space="PSUM") as psB:

            # ---- load persistent weights ----
            def ld(src_ap, shape, d, tag):
                t = sw.tile(shape, d, tag=tag, name=tag)
                nc.sync.dma_start(out=t[:], in_=src_ap)
                return t

            ident = sw.tile([P, P], dt.float32, tag="ident")
            make_identity(nc, ident[:])
            eps_t = sw.tile([1, 1], dt.float32, tag="eps")
            nc.vector.memset(eps_t[:], EPS)
            ones1 = sw.tile([1, P], dt.float32, tag="ones1")
            nc.vector.memset(ones1[:], 1.0)
            identb = sw.tile([P, P], dt.bfloat16, tag="identb")
            make_identity(nc, identb[:])
            ones_sb = ld(onesb[:], [P, 1], dt.bfloat16, "ones")

            wq_t, wk_t, wv_t, wo_t, w1_t, w2_t = [], [], [], [], [], []
            bq_t, bk_t, bo_t, b1_t, b2_t, ln_t = [], [], [], [], [], []
            for p4 in range(4):
                wq_t.append(ld(wqT[p4], [P, 4, P], dt.bfloat16, f"wq{p4}"))
                wk_t.append(ld(wkT[p4], [P, 4, P], dt.bfloat16, f"wk{p4}"))
                wv_t.append(ld(wvT[p4], [P, D], dt.bfloat16, f"wv{p4}"))
                wo_t.append(ld(woT[p4], [P, D], dt.bfloat16, f"wo{p4}"))
                w1_t.append(ld(w1T[p4], [P, DFC, P], dt.bfloat16, f"w1{p4}"))
                w2_t.append(ld(w2T[p4], [P, DFC, P], dt.bfloat16, f"w2{p4}"))
                bq_t.append(ld(bq[p4], [P, 4], dt.float32, f"bq{p4}"))
                bk_t.append(ld(bk[p4], [P, 4], dt.float32, f"bk{p4}"))
                bo_t.append(ld(boe[p4], [P, 1], dt.float32, f"bo{p4}"))
                b1_t.append(ld(b1[p4], [P, DFC], dt.float32, f"b1{p4}"))
                b2_t.append(ld(b2[p4], [P, 1], dt.float32, f"b2{p4}"))
                ln_t.append(ld(lnp[p4], [P, 4], dt.float32, f"ln{p4}"))
            wlin_t = ld(wlinT[:], [P, 2, HID], dt.bfloat16, "wlin")
            blin_t = ld(blin[:], [P, 4], dt.float32, "blin")
            wtab_t = ld(wtabT[:], [P, 4, VSH_T], dt.bfloat16, "wtab")

            # ---- embeddings: gather + transpose into xT[enc] [128, 3200] bf16 ----
            xT = [sw.tile([P, NSEQ * S], dt.bfloat16, tag=f"xT{e}", name=f"xT{e}") for e in range(2)]
            for enc, (emb, idxs) in enumerate(((temb, tidx), (iemb, iidx))):
                idx_sb = sw.tile([P, NCHUNK], dt.int32, tag=f"idx{enc}")
                nc.sync.dma_start(out=idx_sb[:], in_=idxs[:])
                for c in range(NCHUNK):
                    gx = ss.tile([P, D], dt.float32, tag="gx")
                    nc.gpsimd.indirect_dma_start(
                        out=gx[:], out_offset=None, in_=emb[:],
                        in_offset=bass.IndirectOffsetOnAxis(ap=idx_sb[:, c:c + 1], axis=0))
                    xp = psA.tile([P, P], dt.float32, tag="ps")
                    nc.tensor.transpose(out=xp[:], in_=gx[:], identity=ident[:])
                    nc.scalar.copy(out=xT[enc][:, c * P:(c + 1) * P], in_=xp[:])

            # means accumulators [128, BL] f32
            mean_t = [sw.tile([P, BL], dt.float32, tag=f"mean{e}", name=f"mean{e}") for e in range(2)]

            # ---- transformer ----
            for enc in range(2):
                x = xT[enc]
                for l in range(LAYERS):
                    p4 = enc * 2 + l
                    last = (l == LAYERS - 1)
                    for pair in range(NPAIR):
                        c0 = pair * 2 * S
                        W = 2 * S  # 400
                        xs = x[:, c0:c0 + W]
                        # qT, kT in head-padded tiles: tile ti holds heads 2ti
                        # (partitions 0:16) and 2ti+1 (partitions 64:80)
                        qTl, kTl = [], []
                        for ti in range(4):
                            q_ps = psA.tile([P, W], dt.float32, tag="ps")
                            nc.tensor.matmul(out=q_ps[:], lhsT=wq_t[p4][:, ti, :],
                                             rhs=xs, start=True, stop=True)
                            qT = ss.tile([P, W], dt.bfloat16, tag=f"qT{ti}",
                                         name=f"qT{ti}")
                            nc.scalar.activation(out=qT[:], in_=q_ps[:],
                                                 func=AF.Identity,
                                                 bias=bq_t[p4][:, ti:ti + 1], scale=1.0)
                            qTl.append(qT)
                            k_ps = psA.tile([P, W], dt.float32, tag="ps")
                            nc.tensor.matmul(out=k_ps[:], lhsT=wk_t[p4][:, ti, :],
                                             rhs=xs, start=True, stop=True)
                            kT = ss.tile([P, W], dt.bfloat16, tag=f"kT{ti}",
                                         name=f"kT{ti}")
                            nc.scalar.activation(out=kT[:], in_=k_ps[:],
                                                 func=AF.Identity,
                                                 bias=bk_t[p4][:, ti:ti + 1], scale=1.0)
                            kTl.append(kT)
                        # v natural layout + ones col: va[j][kc] [<=128, 8, 18]
                        va = {}
                        for j in range(2):
                            for kc, (ko, kn) in enumerate(KCH):
                                v_ps = psA.tile([P, D], dt.float32, tag="ps")
                                nc.tensor.matmul(
                                    out=v_ps[:kn, :],
                                    lhsT=x[:, c0 + j * S + ko: c0 + j * S + ko + kn],
                                    rhs=wv_t[p4][:], start=True, stop=True)
                                vt = ss.tile([P, H, HD + 2], dt.bfloat16,
                                             tag=f"va{j}{kc}", name=f"va{j}{kc}")
                                nc.vector.tensor_copy(
                                    out=vt[:kn, :, 0:HD],
                                    in_=v_ps[:kn, :].rearrange("p (h d) -> p h d", h=H))
                                nc.vector.memset(vt[:kn, :, HD:HD + 1], 1.0)
                                va[(j, kc)] = vt
                        # attention per (seq, head); o assembled in NATURAL layout
                        # (heads along free dim), normalized, then transposed.
                        oT = ss.tile([P, W], dt.bfloat16, tag="oT")
                        for j in range(2):
                            el = {}
                            for h in range(H):
                                ti, hb = h // 2, 64 * (h % 2)
                                hps = slice(hb, hb + HD)
                                for kc, (ko, kn) in enumerate(KCH):
                                    sc = psA.tile([P, S], dt.float32, tag="ps")
                                    nc.tensor.matmul(
                                        out=sc[:kn, :],
                                        lhsT=kTl[ti][hps, j * S + ko: j * S + ko + kn],
                                        rhs=qTl[ti][hps, j * S: (j + 1) * S],
                                        start=True, stop=True)
                                    e = ss.tile([P, S], dt.bfloat16, tag=f"e{h}{kc}",
                                                name=f"e{h}{kc}")
                                    nc.scalar.activation(out=e[:kn, :], in_=sc[:kn, :],
                                                         func=AF.Exp, scale=0.25)
                                    el[(h, kc)] = e
                            # o_buf[q, 17h:17h+17] = sum_k e_h[k,q] * [v_h | 1][k,:]
                            for qc, (qo, qn) in enumerate(KCH):
                                ob = psA.tile([P, H * (HD + 1)], dt.float32, tag="ps")
                                for h in range(H):
                                    for kc, (ko, kn) in enumerate(KCH):
                                        nc.tensor.matmul(
                                            out=ob[:qn, h * (HD + 1):(h + 1) * (HD + 1)],
                                            lhsT=el[(h, kc)][:kn, qo:qo + qn],
                                            rhs=va[(j, kc)][:kn, h, 0:HD + 1],
                                            start=(kc == 0), stop=(kc == 1))
                                # extract denominators (free-dim strided), normalize
                                rcq = ss.tile([P, H], dt.float32, tag="rcq")
                                dnq = ss.tile([P, H], dt.float32, tag="dnq")
                                nc.vector.tensor_copy(
                                    out=dnq[:qn, :],
                                    in_=ob[:qn, :].rearrange(
                                        "p (h d) -> p h d", h=H)[:, :, HD:HD + 1].squeeze())
                                nc.vector.reciprocal(out=rcq[:qn, :], in_=dnq[:qn, :])
                                onat = ss.tile([P, D], dt.bfloat16, tag="onat")
                                nc.vector.tensor_tensor(
                                    out=onat[:qn, :].rearrange("p (h d) -> p h d", h=H),
                                    in0=ob[:qn, :].rearrange(
                                        "p (h d) -> p h d", h=H)[:, :, 0:HD],
                                    in1=rcq[:qn, :].unsqueeze(2).broadcast_to(
                                        [qn, H, HD]),
                                    op=OP.mult)
                                # transpose back to oT columns
                                otp = psB.tile([P, P], dt.bfloat16, tag="pst")
                                nc.tensor.transpose(out=otp[:, :qn], in_=onat[:qn, :],
                                                    identity=identb[:qn, :qn])
                                nc.scalar.copy(out=oT[:, j * S + qo: j * S + qo + qn],
                                               in_=otp[:, :qn])
                        # Wo + residual
                        y_ps = psA.tile([P, W], dt.float32, tag="ps")
                        nc.tensor.matmul(out=y_ps[:], lhsT=wo_t[p4][:], rhs=oT[:],
                                         start=True, stop=True)
                        yb = ss.tile([P, W], dt.bfloat16, tag="yb")
                        nc.scalar.activation(out=yb[:], in_=y_ps[:], func=AF.Identity,
                                             bias=bo_t[p4][:, 0:1], scale=1.0)
                        x1 = ss.tile([P, W], dt.bfloat16, tag="x1")
                        nc.vector.tensor_tensor(out=x1[:], in0=xs, in1=yb[:], op=OP.add)

                        def layer_norm(xin, g_ap, b_ap, out_ap, accum_cols=None):
                            # stats over partition dim via ones-matmul
                            sq = ss.tile([P, W], dt.bfloat16, tag="sq")
                            nc.vector.tensor_tensor(out=sq[:], in0=xin[:], in1=xin[:],
                                                    op=OP.mult)
                            s1 = psA.tile([1, W], dt.float32, tag="ps")
                            nc.tensor.matmul(out=s1[:], lhsT=ones_sb[:], rhs=xin[:],
                                             start=True, stop=True)
                            s2 = psA.tile([1, W], dt.float32, tag="ps")
                            nc.tensor.matmul(out=s2[:], lhsT=ones_sb[:], rhs=sq[:],
                                             start=True, stop=True)
                            m = ss.tile([1, W], dt.float32, tag="m")
                            nc.scalar.activation(out=m[:], in_=s1[:], func=AF.Copy,
                                                 scale=1.0 / D)
                            msq = ss.tile([1, W], dt.float32, tag="msq")
                            nc.scalar.activation(out=msq[:], in_=s2[:], func=AF.Copy,
                                                 scale=1.0 / D)
                            m2 = ss.tile([1, W], dt.float32, tag="m2")
                            nc.vector.tensor_tensor(out=m2[:], in0=m[:], in1=m[:],
                                                    op=OP.mult)
                            var = ss.tile([1, W], dt.float32, tag="var")
                            nc.vector.tensor_tensor(out=var[:], in0=msq[:], in1=m2[:],
                                                    op=OP.subtract)
                            std = ss.tile([1, W], dt.float32, tag="std")
                            nc.scalar.activation(out=std[:], in_=var[:], func=AF.Sqrt,
                                                 bias=eps_t[0:1, 0:1], scale=1.0)
                            rstd = ss.tile([1, W], dt.float32, tag="rstd")
                            nc.vector.reciprocal(out=rstd[:], in_=std[:])
                            mrep = psA.tile([P, W], dt.float32, tag="ps")
                            nc.tensor.matmul(out=mrep[:], lhsT=ones1[:], rhs=m[:],
                                             start=True, stop=True)
                            rrep = psA.tile([P, W], dt.float32, tag="ps")
                            nc.tensor.matmul(out=rrep[:], lhsT=ones1[:], rhs=rstd[:],
                                             start=True, stop=True)
                            t0 = ss.tile([P, W], dt.bfloat16, tag="t0")
                            nc.vector.tensor_tensor(out=t0[:], in0=xin[:],
                                                    in1=mrep[:], op=OP.subtract)
                            t1 = ss.tile([P, W], dt.bfloat16, tag="t1")
                            nc.vector.tensor_tensor(out=t1[:], in0=t0[:],
                                                    in1=rrep[:], op=OP.mult)
                            if accum_cols is None:
                                nc.scalar.activation(out=out_ap, in_=t1[:],
                                                     func=AF.Identity, bias=b_ap,
                                                     scale=g_ap)
                            else:
                                for j2 in range(2):
                                    nc.scalar.activation(
                                        out=out_ap[:, j2 * S:(j2 + 1) * S],
                                        in_=t1[:, j2 * S:(j2 + 1) * S],
                                        func=AF.Identity, bias=b_ap, scale=g_ap,
                                        accum_out=accum_cols[j2])

                        xn = ss.tile([P, W], dt.bfloat16, tag="xn")
                        layer_norm(x1, ln_t[p4][:, 0:1], ln_t[p4][:, 1:2], xn[:])
                        # FFN
                        f2_ps = psB.tile([P, W], dt.float32, tag="acc")
                        for fc in range(DFC):
                            f1_ps = psA.tile([P, W], dt.float32, tag="ps")
                            nc.tensor.matmul(out=f1_ps[:], lhsT=w1_t[p4][:, fc, :],
                                             rhs=xn[:], start=True, stop=True)
                            f1b = ss.tile([P, W], dt.bfloat16, tag="f1b")
                            nc.scalar.activation(out=f1b[:], in_=f1_ps[:], func=AF.Relu,
                                                 bias=b1_t[p4][:, fc:fc + 1], scale=1.0)
                            nc.tensor.matmul(out=f2_ps[:], lhsT=w2_t[p4][:, fc, :],
                                             rhs=f1b[:], start=(fc == 0),
                                             stop=(fc == DFC - 1))
                        fb = ss.tile([P, W], dt.bfloat16, tag="fb")
                        nc.scalar.activation(out=fb[:], in_=f2_ps[:], func=AF.Identity,
                                             bias=b2_t[p4][:, 0:1], scale=1.0)
                        x2 = ss.tile([P, W], dt.bfloat16, tag="x2")
                        nc.vector.tensor_tensor(out=x2[:], in0=xn[:], in1=fb[:], op=OP.add)
                        if not last:
                            layer_norm(x2, ln_t[p4][:, 2:3], ln_t[p4][:, 3:4], xs)
                        else:
                            rows = (2 * pair, 2 * pair + 1)
                            layer_norm(
                                x2, ln_t[p4][:, 2:3], ln_t[p4][:, 3:4], xs,
                                accum_cols=[mean_t[enc][:, r:r + 1] for r in rows])

            # ---- MLP head: hT [512, BL] -> cc_in ----
            mb = []
            for e in range(2):
                t = ss.tile([P, BL], dt.bfloat16, tag=f"mb{e}", name=f"mb{e}")
                nc.vector.tensor_copy(out=t[:], in_=mean_t[e][:])
                mb.append(t)
            hloc = ss.tile([P, 4, BL], dt.bfloat16, tag="hloc")
            for m in range(4):
                h_ps = psA.tile([P, BL], dt.float32, tag="ps")
                for c in range(2):
                    nc.tensor.matmul(out=h_ps[:],
                                     lhsT=wlin_t[:, c, m * P:(m + 1) * P],
                                     rhs=mb[c][:], start=(c == 0), stop=(c == 1))
                nc.scalar.activation(out=hloc[:, m, :], in_=h_ps[:], func=AF.Relu,
                                     bias=blin_t[:, m:m + 1], scale=1.0 / S)
            nc.sync.dma_start(out=cc_in[:], in_=hloc[:])
            if single:
                # timing-only variant: no collective (TimelineSim is 1-core)
                for r in range(NCORE):
                    nc.sync.dma_start(out=cc_out[r], in_=cc_in[:])
            else:
                nc.gpsimd.collective_compute(
                    "AllGather", OP.bypass,
                    replica_groups=[list(range(NCORE))],
                    ins=[cc_in[:]], outs=[cc_out[:]])
            hT = sw.tile([P, 4, B], dt.bfloat16, tag="hT")
            for r in range(NCORE):
                for kk in range(4):
                    nc.sync.dma_start(out=hT[:, kk, r * BL:(r + 1) * BL],
                                      in_=cc_out[r, :, kk, :])

            # ---- projections ----
            def proj(nvt, vtw, w_tiles_fn, vec_dram, out_dram):
                for vt_i in range(nvt):
                    v0 = vt_i * vtw
                    vb = sp.tile([1, vtw], dt.float32, tag="vb", name="vb")
                    nc.sync.dma_start(out=vb[:], in_=vec_dram[0:1, v0:v0 + vtw])
                    vl = sp.tile([1, vtw], dt.float32, tag="vl", name="vl")
                    nc.sync.dma_start(out=vl[:], in_=vec_dram[1:2, v0:v0 + vtw])
                    vbr_ps = psA.tile([P, vtw], dt.float32, tag="ps")
                    nc.tensor.matmul(out=vbr_ps[:], lhsT=ones1[:], rhs=vb[:],
                                     start=True, stop=True)
                    vbr = sp.tile([P, vtw], dt.float32, tag="vbr", name="vbr")
                    nc.scalar.copy(out=vbr[:], in_=vbr_ps[:])
                    vlr_ps = psA.tile([P, vtw], dt.float32, tag="ps")
                    nc.tensor.matmul(out=vlr_ps[:], lhsT=ones1[:], rhs=vl[:],
                                     start=True, stop=True)
                    vlr = sp.tile([P, vtw], dt.float32, tag="vlr", name="vlr")
                    nc.scalar.copy(out=vlr[:], in_=vlr_ps[:])
                    bp = psA.tile([P, vtw], dt.float32, tag="ps")
                    for kk in range(4):
                        nc.tensor.matmul(out=bp[:], lhsT=hT[:, kk, :],
                                         rhs=w_tiles_fn(kk, v0, vtw),
                                         start=(kk == 0), stop=(kk == 3))
                    prev = None
                    for t in range(TOUT):
                        ot = sp.tile([P, vtw], dt.float32, tag=f"o{t}", name=f"o{t}")
                        nc.vector.tensor_tensor(
                            out=ot[:], in0=(bp[:] if t == 0 else prev[:]),
                            in1=(vbr if t == 0 else vlr)[:, :], op=OP.add)
                        nc.sync.dma_start(out=out_dram[:, t, v0:v0 + vtw], in_=ot[:])
                        prev = ot

            def wtab_fn(kk, v0, vtw):
                return wtab_t[:, kk, v0:v0 + vtw].squeeze()

            def widx_fn(kk, v0, vtw):
                wt = sp.tile([P, vtw], dt.bfloat16, tag="wvt")
                nc.sync.dma_start(out=wt[:], in_=widxT[kk * P:(kk + 1) * P, v0:v0 + vtw])
                return wt[:]

            proj(1, VSH_T, wtab_fn, tvec, otab)
            proj(NVT, VT, widx_fn, ivec, oidx)

    nc.compile()
    return nc


def _prep(inputs):
    f32 = np.float32
    bf = ml_dtypes.bfloat16
    g = {k: np.asarray(v) for k, v in inputs.items()}

    Wqkv, bqkv = g["Wqkv"], g["bqkv"]
    Wo, bo = g["Wo"], g["bo"]
    W1, b1_, W2, b2_ = g["W1"], g["b1"], g["W2"], g["b2"]

    wqT = np.zeros((4, P, 4, P), f32); wkT = np.zeros((4, P, 4, P), f32)
    wvT = np.zeros((4, P, D), f32); woT = np.zeros((4, P, D), f32)
    w1T = np.zeros((4, P, DFC, P), f32); w2T = np.zeros((4, P, DFC, P), f32)
    bq = np.zeros((4, P, 4), f32); bk = np.zeros((4, P, 4), f32)
    boe = np.zeros((4, P, 1), f32); b1h = np.zeros((4, P, DFC), f32)
    b2h = np.zeros((4, P, 1), f32); lnp = np.zeros((4, P, 4), f32)
    for e in range(2):
        for l in range(LAYERS):
            p4 = e * 2 + l
            WqTf = Wqkv[e, l, 0:D, :].T       # [in-D, out-D]
            WkTf = Wqkv[e, l, D:2 * D, :].T
            for ti in range(4):
                for s2, h in ((0, 2 * ti), (64, 2 * ti + 1)):
                    wqT[p4, :, ti, s2:s2 + HD] = WqTf[:, h * HD:(h + 1) * HD]
                    wkT[p4, :, ti, s2:s2 + HD] = WkTf[:, h * HD:(h + 1) * HD]
                    bq[p4, s2:s2 + HD, ti] = bqkv[e, l, h * HD:(h + 1) * HD]
                    bk[p4, s2:s2 + HD, ti] = bqkv[e, l, D + h * HD:D + (h + 1) * HD]
            wvT[p4] = Wqkv[e, l, 2 * D:3 * D, :].T
            woT[p4] = Wo[e, l].T
            w1T[p4] = W1[e, l].T.reshape(P, DFC, P)
            w2T[p4] = W2[e, l].T.reshape(DFC, P, P).transpose(1, 0, 2)
            bv = bqkv[e, l, 2 * D:3 * D]
            boe[p4, :, 0] = bo[e, l] + Wo[e, l] @ bv
            b1h[p4] = b1_[e, l].reshape(DFC, P).T
            b2h[p4, :, 0] = b2_[e, l]
            lnp[p4, :, 0] = g["ln1_g"][e, l]; lnp[p4, :, 1] = g["ln1_b"][e, l]
            lnp[p4, :, 2] = g["ln2_g"][e, l]; lnp[p4, :, 3] = g["ln2_b"][e, l]

    wlinT = g["Wlin"].T.reshape(2, P, HID).transpose(1, 0, 2).copy()  # [128,2,512]
    blin = g["blin"].reshape(4, P).T                  # [128, 4]
    Wtab, btab = g["Wtab"], g["btab"]
    Widx, bidx = g["Widx"], g["bidx"]
    onesb = np.ones((P, 1), f32)

    in_maps = []
    for c in range(NCORE):
        rows = slice(c * BL, (c + 1) * BL)
        tids = g["table_id_seq"][rows].reshape(-1)     # [3200]
        iids = g["idx_id_seq"][rows].reshape(-1)
        tsh = slice(c * VSH_T, (c + 1) * VSH_T)
        ish = slice(c * VSH_I, (c + 1) * VSH_I)
        WtabS, WidxS = Wtab[tsh], Widx[ish]
        tvec = np.stack([btab[tsh], WtabS[:, HID]]).astype(f32)
        ivec = np.stack([bidx[ish], WidxS[:, HID]]).astype(f32)
        in_maps.append({
            "temb": g["table_embed"].astype(f32),
            "iemb": g["idx_embed"].astype(f32),
            "tidx": tids.reshape(NCHUNK, P).T.copy().astype(np.int32),
            "iidx": iids.reshape(NCHUNK, P).T.copy().astype(np.int32),
            "wqT": wqT.astype(bf), "wkT": wkT.astype(bf),
            "wvT": wvT.astype(bf), "woT": woT.astype(bf),
            "w1T": w1T.astype(bf), "w2T": w2T.astype(bf),
            "bq": bq, "bk": bk, "boe": boe, "b1": b1h, "b2": b2h, "lnp": lnp,
            "wlinT": wlinT.astype(bf), "blin": blin.astype(f32),
            "wtabT": WtabS[:, :HID].T.reshape(4, P, VSH_T).transpose(1, 0, 2).copy().astype(bf),
            "widxT": WidxS[:, :HID].T.copy().astype(bf),
            "tvec": tvec, "ivec": ivec,
            "onesb": onesb.astype(bf),
        })
    return in_maps


def kernel(**inputs):
    if "nc" not in _CACHE:
        _CACHE["nc"] = _build_nc()
    nc = _CACHE["nc"]
    in_maps = _prep(inputs)
    res = run_bass_kernel_spmd(nc, in_maps, core_ids=list(range(NCORE)))
    _CACHE["last_res"] = res
    tab = np.concatenate(
        [res.results[c]["otab"].reshape(B * TOUT, VSH_T) for c in range(NCORE)], axis=1)
    idx = np.concatenate(
        [res.results[c]["oidx"].reshape(B * TOUT, VSH_I) for c in range(NCORE)], axis=1)
    return tab, idx
